# revision 48
# baseline (speedup 1.0000x reference)
"""Trainium2 Bass kernel for nn_DetectionModule (cross-attn + pos/neg expert MLPs).

Self-contained: hardcodes shapes B=32, T=128, R=49, D=512, H=256.
Sharding: pure data-parallel, 4 batches per core across 8 cores.

Key algebraic factorization (avoids materializing [B,T,R,D] = 1.6 GB):
  all_emb @ nw1       = (word@nw1)[t] + (region@nw1)[r]        (broadcast sum)
  einsum(btr,btrd->bd) = sum_t s.sum(r)[t]*word[t] + sum_r s.sum(t)[r]*region[r]

Precision: fp32 everywhere except the neg-expert H pipeline
(wn/rn broadcast-add, tanh, nw2 contraction) which runs in fp16
(HW-measured end-to-end rel err ~1.2e-4 vs jax fp32 reference).

Schedule: engines execute their instruction streams in order, so the
batches are software-pipelined at emission: A1(b)=loads/transposes/attn/
softmax/wn/rn | B1(b-1)=pos path+contraction | A2(b)=H adds+tanh |
B2(b-1)=sigma/mask/reductions/aggs. The nw2 contraction packs 4 psum rows
per bank via tile_position col-groups (zero-padded 32-wide weights) and
ACT evicts them with the sigma-tanh fused; 9 of 49 r-columns' adds run
on GPSIMD in a separate tile to offload the DVE. The head computes choose/softmax/final on a [4, 2*D] packed AGG layout
(no cross-partition DMA hops). Cost-model timeline: ~113us/core (from
184us naive ordering).
"""
import sys

if "/opt/trn_rl_repo" not in sys.path:
    sys.path.insert(0, "/opt/trn_rl_repo")

import numpy as np

import concourse.bass as bass
import concourse.bacc as bacc
import concourse.tile as tile
from concourse import masks, mybir
from concourse.bass_utils import run_bass_kernel_spmd

F32 = mybir.dt.float32
F16 = mybir.dt.float16
U8 = mybir.dt.uint8
I32 = mybir.dt.int32
AF = mybir.ActivationFunctionType
ALU = mybir.AluOpType
AX = mybir.AxisListType

B, T, R, D, H = 32, 128, 49, 512, 256
NCORES = 8
BPC = B // NCORES  # batches per core = 4
NDC = D // 128     # d chunks = 4
NHC = H // 128     # h chunks = 2
RT = R * T         # 6272
# r-blocks for the contraction: 12 blocks of 4 + 1 block of 1 (psum bank = 512 f32)
RBLKS = [(i * 4, 4) for i in range(12)] + [(48, 1)]


def _ap(t, dims, offset=0):
    """AP on tile t keeping its partition dim, custom free dims [[step,count],...]."""
    a = t[...] if not isinstance(t, bass.AP) else t
    return bass.AP(tensor=a.tensor, offset=a.offset + offset, ap=[a.ap[0]] + dims)


def build_nc():
    nc = bacc.Bacc(None, target_bir_lowering=False)

    word_d = nc.dram_tensor("word_emb", [BPC, T, D], F32, kind="ExternalInput")
    region_d = nc.dram_tensor("region_emb", [BPC, R, D], F32, kind="ExternalInput")
    mask_d = nc.dram_tensor("attention_mask", [BPC, T], I32, kind="ExternalInput")
    pw1_d = nc.dram_tensor("pw1", [D, H], F32, kind="ExternalInput")
    pb1_d = nc.dram_tensor("pb1", [H], F32, kind="ExternalInput")
    pw2_d = nc.dram_tensor("pw2", [H, 1], F32, kind="ExternalInput")
    pb2_d = nc.dram_tensor("pb2", [1], F32, kind="ExternalInput")
    nw1_d = nc.dram_tensor("nw1", [D, H], F32, kind="ExternalInput")
    nb1_d = nc.dram_tensor("nb1", [H], F32, kind="ExternalInput")
    nw2_d = nc.dram_tensor("nw2", [H, 1], F32, kind="ExternalInput")
    nb2_d = nc.dram_tensor("nb2", [1], F32, kind="ExternalInput")
    fcw_d = nc.dram_tensor("fcw", [D, 1], F32, kind="ExternalInput")
    fcb_d = nc.dram_tensor("fcb", [1], F32, kind="ExternalInput")
    cw1_d = nc.dram_tensor("cw1", [D, H], F32, kind="ExternalInput")
    cb1_d = nc.dram_tensor("cb1", [H], F32, kind="ExternalInput")
    cw2_d = nc.dram_tensor("cw2", [H, 2], F32, kind="ExternalInput")
    cb2_d = nc.dram_tensor("cb2", [2], F32, kind="ExternalInput")
    logits_d = nc.dram_tensor("logits", [BPC, 2], F32, kind="ExternalOutput")
    pp_d = nc.dram_tensor("path_prob", [BPC, 2], F32, kind="ExternalOutput")

    with tile.TileContext(nc) as tc:
        with (
            tc.tile_pool(name="const", bufs=1) as const,
            tc.tile_pool(name="io", bufs=3) as io,
            tc.tile_pool(name="tr", bufs=3) as tr,
            tc.tile_pool(name="med", bufs=4) as med,
            tc.tile_pool(name="small", bufs=4) as small,
            tc.tile_pool(name="hpool", bufs=6) as hpool,
            tc.tile_pool(name="ps_tr", bufs=2, space="PSUM") as ps_tr,
            tc.tile_pool(name="ps_big", bufs=4, space="PSUM") as ps_big,
            tc.tile_pool(name="ps_row", bufs=2, space="PSUM") as ps_row,
        ):
            # ---------------- early input loads (cut prologue latency) ----------------
            preload = {}
            for b in range(2):
                word_s = io.tile([T, D], F32, tag="word", name=f"word{b}")
                nc.sync.dma_start(out=word_s, in_=word_d[b, :, :])
                region_s = io.tile([R, D], F32, tag="region", name=f"region{b}")
                nc.sync.dma_start(out=region_s, in_=region_d[b, :, :])
                mask_i = small.tile([T, 1], I32, tag="mask_i", name=f"mask_i{b}")
                nc.sync.dma_start(out=mask_i, in_=mask_d[b, :].rearrange("(t o) -> t o", o=1))
                preload[b] = (word_s, region_s, mask_i)

            # ---------------- global constants / params ----------------
            ident = const.tile([128, 128], F32)
            masks.make_identity(nc, ident[:, :])
            ones_col = const.tile([128, 1], F32)
            nc.vector.memset(ones_col, 1.0)
            neginf4 = const.tile([BPC, 1], F32)
            nc.vector.memset(neginf4, -1e9)


            pw1_s = const.tile([128, NDC, H], F32)
            nc.sync.dma_start(out=pw1_s, in_=pw1_d[:, :].rearrange("(c p) h -> p c h", p=128))
            cw1_s = const.tile([128, NDC, H], F32)
            nc.sync.dma_start(out=cw1_s, in_=cw1_d[:, :].rearrange("(c p) h -> p c h", p=128))
            nw1_s = const.tile([128, NDC, H], F32)
            nc.sync.dma_start(out=nw1_s, in_=nw1_d[:, :].rearrange("(c p) h -> p c h", p=128))

            pw2_s = const.tile([128, NHC], F32)
            nc.sync.dma_start(out=pw2_s, in_=pw2_d[:, :].rearrange("(c p) o -> p (c o)", p=128))
            nw2_s = const.tile([128, NHC], F32)
            nc.sync.dma_start(out=nw2_s, in_=nw2_d[:, :].rearrange("(c p) o -> p (c o)", p=128))
            nw2_16 = const.tile([128, NHC], F16)
            nc.vector.tensor_copy(nw2_16, nw2_s)
            nw2pad = const.tile([128, NHC, 32], F16)
            nc.vector.memset(nw2pad, 0.0)
            for hc in range(NHC):
                nc.vector.tensor_copy(nw2pad[:, hc, 0:1], nw2_s[:, hc:hc + 1])
            cw2_s = const.tile([128, NHC, 2], F32)
            nc.sync.dma_start(out=cw2_s, in_=cw2_d[:, :].rearrange("(c p) o -> p c o", p=128))

            pb1c = const.tile([128, NHC], F32)
            nc.sync.dma_start(out=pb1c, in_=pb1_d[:].rearrange("(c p) -> p c", p=128))
            nb1c = const.tile([128, NHC], F32)
            nc.sync.dma_start(out=nb1c, in_=nb1_d[:].rearrange("(c p) -> p c", p=128))
            cb1c = const.tile([128, NHC], F32)
            nc.sync.dma_start(out=cb1c, in_=cb1_d[:].rearrange("(c p) -> p c", p=128))

            # scalar biases broadcast to partitions; pb2h/nb2h = 0.5*bias (sigmoid-via-tanh)
            pb2b = const.tile([128, 1], F32)
            nc.sync.dma_start(out=pb2b, in_=bass.AP(tensor=pb2_d, offset=0, ap=[[0, 128], [1, 1]]))
            pb2h = const.tile([128, 1], F32)
            nc.vector.tensor_scalar_mul(pb2h, pb2b, 0.5)
            nb2b = const.tile([128, 1], F32)
            nc.sync.dma_start(out=nb2b, in_=bass.AP(tensor=nb2_d, offset=0, ap=[[0, 128], [1, 1]]))
            nb2h = const.tile([128, 1], F32)
            nc.vector.tensor_scalar_mul(nb2h, nb2b, 0.5)

            fcw2 = const.tile([BPC, 2 * D], F32)
            nc.sync.dma_start(out=fcw2, in_=bass.AP(tensor=fcw_d, offset=0,
                                                    ap=[[0, BPC], [0, 2], [1, D]]))
            fcb4 = const.tile([BPC, 1], F32)
            nc.sync.dma_start(out=fcb4, in_=bass.AP(tensor=fcb_d, offset=0, ap=[[0, BPC], [1, 1]]))
            cb2b = const.tile([BPC, 2], F32)
            nc.sync.dma_start(out=cb2b, in_=bass.AP(tensor=cb2_d, offset=0, ap=[[0, BPC], [1, 2]]))

            AGG = const.tile([BPC, 2 * D], F32)      # row b = [neg_agg(b) | pos_agg(b)]
            totJ = const.tile([BPC, 1], F32)         # per-batch sum of attn_neg

            # ---------------- per-batch pipeline (2-stage software pipeline) ----------------
            def stageA(b):
                """loads, transposes, attention+softmax, wn/rn, H adds + tanh"""
                ctx = {}
                if b in preload:
                    word_s, region_s, mask_i = preload[b]
                else:
                    word_s = io.tile([T, D], F32, tag="word", name=f"word{b}")
                    nc.sync.dma_start(out=word_s, in_=word_d[b, :, :])
                    region_s = io.tile([R, D], F32, tag="region", name=f"region{b}")
                    nc.sync.dma_start(out=region_s, in_=region_d[b, :, :])
                    mask_i = small.tile([T, 1], I32, tag="mask_i", name=f"mask_i{b}")
                    nc.sync.dma_start(out=mask_i, in_=mask_d[b, :].rearrange("(t o) -> t o", o=1))
                mask_c = small.tile([T, 1], F32, tag="mask_c", name=f"mask_c{b}")
                nc.gpsimd.tensor_copy(mask_c, mask_i)

                wordT32 = tr.tile([128, NDC, T], F32, tag="wordT32", name=f"wordT32_{b}")
                for c in range(NDC):
                    pt = ps_tr.tile([128, T], F32, tag="tr", name=f"ptw{b}_{c}")
                    nc.tensor.transpose(pt, word_s[:, c * 128:(c + 1) * 128], ident[:, :])
                    nc.vector.tensor_copy(wordT32[:, c, :], pt)
                regionT32 = tr.tile([128, NDC, R], F32, tag="regionT32", name=f"regionT32_{b}")
                for c in range(NDC):
                    pt = ps_tr.tile([128, R], F32, tag="tr", name=f"ptr{b}_{c}")
                    nc.tensor.transpose(pt, region_s[:, c * 128:(c + 1) * 128], ident[:R, :R])
                    nc.vector.tensor_copy(regionT32[:, c, :], pt)

                # attention [T, R] fp32
                ps_attn = ps_big.tile([T, R], F32, tag="big", name=f"psattn{b}")
                for c in range(NDC):
                    nc.tensor.matmul(ps_attn, wordT32[:, c, :], regionT32[:, c, :],
                                     start=(c == 0), stop=(c == NDC - 1))
                attn_s = small.tile([T, R], F32, tag="attn", name=f"attn{b}")
                nc.vector.tensor_copy(attn_s, ps_attn)
                negf = small.tile([T, R], F32, tag="negf", name=f"negf{b}")
                nc.gpsimd.tensor_scalar(negf, attn_s, 0.1, None, op0=ALU.is_gt)
                negu = small.tile([T, R], U8, tag="negu", name=f"negu{b}")
                nc.gpsimd.tensor_scalar(negu, attn_s, 0.1, None, op0=ALU.is_gt)
                rowneg = small.tile([T, 1], F32, tag="rowneg", name=f"rowneg{b}")
                nc.vector.reduce_sum(rowneg, negf, axis=AX.X)
                m4 = small.tile([T, R], F32, tag="m4", name=f"m4_{b}")
                nc.gpsimd.tensor_scalar_mul(m4, attn_s, 4.0)
                msk = small.tile([T, R], F32, tag="msk", name=f"msk{b}")
                nc.vector.memset(msk, -4e9)
                nc.vector.copy_predicated(msk, negu, m4)
                mx = small.tile([T, 1], F32, tag="mx", name=f"mx{b}")
                nc.vector.tensor_reduce(mx, msk, axis=AX.X, op=ALU.max, negate=True)
                ex = small.tile([T, R], F32, tag="ex", name=f"ex{b}")
                nc.scalar.activation(out=ex, in_=msk, func=AF.Exp, bias=mx[:, :], scale=1.0)
                sume = small.tile([T, 1], F32, tag="sume", name=f"sume{b}")
                nc.vector.reduce_sum(sume, ex, axis=AX.X)
                rec = small.tile([T, 1], F32, tag="rec", name=f"rec{b}")
                nc.vector.reciprocal(rec, sume)
                attn_pos = small.tile([T, R], F32, tag="attn_pos", name=f"attn_pos{b}")
                nc.gpsimd.tensor_scalar_mul(attn_pos, ex, rec[:, :])

                # wn/rn (fp32 matmuls, fp16/fp32 evictions)
                wnT16 = med.tile([128, NHC, T], F16, tag="wnT16", name=f"wnT16_{b}")
                rnT = med.tile([128, NHC, R], F32, tag="rnT", name=f"rnT{b}")
                for hc in range(NHC):
                    pw = ps_big.tile([128, T], F32, tag="big", name=f"pswn{b}_{hc}")
                    for c in range(NDC):
                        nc.tensor.matmul(pw, nw1_s[:, c, hc * 128:(hc + 1) * 128], wordT32[:, c, :],
                                         start=(c == 0), stop=(c == NDC - 1))
                    nc.vector.tensor_copy(wnT16[:, hc, :], pw)
                    pr = ps_tr.tile([128, R], F32, tag="tr", name=f"psrn{b}_{hc}")
                    for c in range(NDC):
                        nc.tensor.matmul(pr, nw1_s[:, c, hc * 128:(hc + 1) * 128], regionT32[:, c, :],
                                         start=(c == 0), stop=(c == NDC - 1))
                    nc.vector.tensor_scalar(rnT[:, hc, :], pr, nb1c[:, hc:hc + 1], None, op0=ALU.add)

                # H16[h, r*T+t] = tanh(wnT16[h,t] + rnT[h,r])  (emitted in stageA2)
                H16 = []
                H16p = []
                for _hc in range(NHC):
                    Htile = hpool.tile([128, 40 * T], F16, tag="H", name=f"H{b}_{_hc}")
                    H16.append(Htile)
                    Hptile = hpool.tile([128, 9 * T], F16, tag="Hp", name=f"Hp{b}_{_hc}")
                    H16p.append(Hptile)

                ctx.update(word_s=word_s, region_s=region_s, mask_c=mask_c, negf=negf,
                           rowneg=rowneg, attn_pos=attn_pos, H16=H16, H16p=H16p,
                           wnT16=wnT16, rnT=rnT)
                return ctx

            def stageA2(b, ctx, piecewise=False):
                H16, H16p, wnT16, rnT = ctx["H16"], ctx["H16p"], ctx["wnT16"], ctx["rnT"]
                # Pool adds first (big broadcast TT per h-chunk; runs alongside DVE)
                for hc in range(NHC):
                    for r in range(40, 49):
                        nc.gpsimd.tensor_scalar(
                            H16p[hc][:, (r - 40) * T:(r - 39) * T], wnT16[:, hc, :],
                            rnT[:, hc, r:r + 1], None, op0=ALU.add)
                for hc in range(NHC):
                    for r in range(40):
                        nc.vector.tensor_scalar(
                            H16[hc][:, r * T:(r + 1) * T], wnT16[:, hc, :],
                            rnT[:, hc, r:r + 1], None, op0=ALU.add)
                    nc.scalar.activation(out=H16[hc][:, :], in_=H16[hc][:, :], func=AF.Tanh)
                    nc.scalar.activation(out=H16p[hc][:, :], in_=H16p[hc][:, :], func=AF.Tanh)

            def stageB(b, ctx):
                """pos path, contraction, postproc, aggregations"""
                word_s, region_s, mask_c = ctx["word_s"], ctx["region_s"], ctx["mask_c"]
                negf, rowneg, attn_pos = ctx["negf"], ctx["rowneg"], ctx["attn_pos"]
                H16, H16p = ctx["H16"], ctx["H16p"]

                # pos path
                ps_apT = ps_tr.tile([R, T], F32, tag="tr", name=f"psapT{b}")
                nc.tensor.transpose(ps_apT, attn_pos[:, :], ident[:, :])
                apT = small.tile([R, T], F32, tag="apT", name=f"apT{b}")
                nc.vector.tensor_copy(apT, ps_apT)
                ps_pos = ps_big.tile([T, D], F32, tag="big", name=f"pspos{b}")
                nc.tensor.matmul(ps_pos, apT[:, :], region_s[:, :], start=True, stop=True)
                pos_s = med.tile([T, D], F32, tag="pos", name=f"pos{b}")
                nc.vector.tensor_tensor(pos_s, ps_pos, word_s, op=ALU.add)
                posT = tr.tile([128, NDC, T], F32, tag="posT", name=f"posT{b}")
                for c in range(NDC):
                    pt = ps_tr.tile([128, T], F32, tag="tr", name=f"ptp{b}_{c}")
                    nc.tensor.transpose(pt, pos_s[:, c * 128:(c + 1) * 128], ident[:, :])
                    nc.vector.tensor_copy(posT[:, c, :], pt)
                hpT = med.tile([128, NHC, T], F32, tag="hpT", name=f"hpT{b}")
                for hc in range(NHC):
                    ph = ps_big.tile([128, T], F32, tag="big", name=f"pshp{b}_{hc}")
                    for c in range(NDC):
                        nc.tensor.matmul(ph, pw1_s[:, c, hc * 128:(hc + 1) * 128], posT[:, c, :],
                                         start=(c == 0), stop=(c == NDC - 1))
                    nc.scalar.activation(out=hpT[:, hc, :], in_=ph, func=AF.Tanh,
                                         bias=pb1c[:, hc:hc + 1], scale=1.0)
                ps_sp = ps_row.tile([T, 1], F32, tag="row", name=f"pssp{b}")
                for hc in range(NHC):
                    nc.tensor.matmul(ps_sp, hpT[:, hc, :], pw2_s[:, hc:hc + 1],
                                     start=(hc == 0), stop=(hc == NHC - 1))
                sct = small.tile([T, 1], F32, tag="sct", name=f"sct{b}")
                nc.scalar.activation(out=sct, in_=ps_sp, func=AF.Tanh,
                                     bias=pb2h[:, :], scale=0.5)
                score = small.tile([T, 1], F32, tag="score", name=f"score{b}")
                nc.gpsimd.tensor_scalar(score, sct, 0.5, 0.5, op0=ALU.mult, op1=ALU.add)
                ind = small.tile([T, 1], F32, tag="ind", name=f"ind{b}")
                nc.gpsimd.tensor_scalar(ind, rowneg, 0.5, None, op0=ALU.is_gt)
                indm = small.tile([T, 1], F32, tag="indm", name=f"indm{b}")
                nc.gpsimd.tensor_tensor(indm, ind, mask_c, op=ALU.mult)
                score_m = small.tile([T, 1], F32, tag="score_m", name=f"score_m{b}")
                nc.gpsimd.tensor_tensor(score_m, score, indm, op=ALU.mult)
                ps_pagg = ps_row.tile([1, D], F32, tag="row", name=f"pspagg{b}")
                nc.tensor.matmul(ps_pagg, score_m[:, :], pos_s[:, :], start=True, stop=True)
                pagg_row = small.tile([1, D], F32, tag="pagg_row", name=f"pagg{b}")
                nc.vector.tensor_copy(pagg_row, ps_pagg)
                nc.sync.dma_start(out=AGG[b:b + 1, D:2 * D], in_=pagg_row)

                # contraction via col groups; ACT evicts with fused sigma-tanh
                sgT = small.tile([R, T], F32, tag="sgT", name=f"sgT{b}")
                for g in range(3):
                    psg = ps_big.tile([128, 4 * T], F32, tag="big", name=f"psg{b}_{g}")
                    for j in range(4):
                        r0 = 16 * g + 4 * j
                        for hc in range(NHC):
                            if r0 < 40:
                                rhs = H16[hc][:, r0 * T:(r0 + 4) * T]
                            else:
                                rhs = H16p[hc][:, (r0 - 40) * T:(r0 - 36) * T]
                            nc.tensor.matmul(psg[32 * j:32 * j + 32, :], nw2pad[:, hc, :],
                                             rhs,
                                             start=(hc == 0), stop=(hc == NHC - 1),
                                             tile_position=(0, 32 * j))
                    evg = small.tile([128, 4 * T], F32, tag="evg", name=f"evg{b}_{g}")
                    nc.scalar.activation(out=evg, in_=psg, func=AF.Tanh,
                                         bias=nb2h[:, :], scale=0.5)
                    src = bass.AP(tensor=evg[...].tensor, offset=evg[...].offset,
                                  ap=[[4 * T * 32, 4], [1, 4 * T]])
                    nc.sync.dma_start(out=sgT[16 * g:16 * g + 16, :], in_=src)
                prow = ps_row.tile([1, T], F32, tag="row", name=f"prow{b}")
                for hc in range(NHC):
                    nc.tensor.matmul(prow[0:1, :], nw2_16[:, hc:hc + 1],
                                     H16p[hc][:, 8 * T:9 * T],
                                     start=(hc == 0), stop=(hc == NHC - 1))
                tail = small.tile([1, T], F32, tag="tailrow", name=f"tail{b}")
                nc.scalar.activation(out=tail, in_=prow, func=AF.Tanh,
                                     bias=nb2h[0:1, :], scale=0.5)
                nc.sync.dma_start(out=sgT[48:49, :], in_=tail[0:1, :])
                ctx["sgT"] = sgT

            def stageB2(b, ctx):
                word_s, region_s, mask_c = ctx["word_s"], ctx["region_s"], ctx["mask_c"]
                negf, rowneg = ctx["negf"], ctx["rowneg"]
                sgT = ctx["sgT"]
                # sigma finishing + mask + reductions in [R, T]
                sg = small.tile([R, T], F32, tag="sg", name=f"sg{b}")
                nc.gpsimd.tensor_scalar(sg, sgT, 0.5, 0.5, op0=ALU.mult, op1=ALU.add)
                nmf = small.tile([T, R], F32, tag="nmf", name=f"nmf{b}")
                nc.gpsimd.tensor_scalar(nmf, negf, -1.0, 1.0, op0=ALU.mult, op1=ALU.add)
                nmf2 = small.tile([T, R], F32, tag="nmf2", name=f"nmf2_{b}")
                nc.gpsimd.tensor_scalar_mul(nmf2, nmf, mask_c[:, :])
                ps_nm = ps_tr.tile([R, T], F32, tag="tr", name=f"psnm{b}")
                nc.tensor.transpose(ps_nm, nmf2[:, :], ident[:, :])
                nm_T = small.tile([R, T], F32, tag="nm_T", name=f"nm_T{b}")
                nc.vector.tensor_copy(nm_T, ps_nm)
                s_fin = small.tile([R, T], F32, tag="s_fin", name=f"s_fin{b}")
                nc.vector.tensor_tensor(s_fin, sg, nm_T, op=ALU.mult)
                sr_s = small.tile([R, 1], F32, tag="sr_s", name=f"sr{b}")
                nc.vector.reduce_sum(sr_s, s_fin, axis=AX.X)
                ps_sw = ps_row.tile([T, 1], F32, tag="row", name=f"pssw{b}")
                nc.tensor.matmul(ps_sw, s_fin[:, :], ones_col[:R, :], start=True, stop=True)
                sw = small.tile([T, 1], F32, tag="sw", name=f"sw{b}")
                nc.vector.tensor_copy(sw, ps_sw)
                ps_nagg = ps_row.tile([1, D], F32, tag="row", name=f"psnagg{b}")
                nc.tensor.matmul(ps_nagg, sw[:, :], word_s[:, :], start=True, stop=False)
                nc.tensor.matmul(ps_nagg, sr_s[:, :], region_s[:, :], start=False, stop=True)
                nagg_row = small.tile([1, D], F32, tag="nagg_row", name=f"nagg{b}")
                nc.vector.tensor_copy(nagg_row, ps_nagg)
                nc.sync.dma_start(out=AGG[b:b + 1, 0:D], in_=nagg_row)

                ps_tot = ps_row.tile([1, 1], F32, tag="row", name=f"pstot{b}")
                nc.tensor.matmul(ps_tot, rowneg[:, :], ones_col[:, :], start=True, stop=True)
                tot1 = small.tile([1, 1], F32, tag="tot1", name=f"tot1_{b}")
                nc.vector.tensor_copy(tot1, ps_tot)
                nc.sync.dma_start(out=totJ[b:b + 1, :], in_=tot1[0:1, :])

            # pipelined emission: A1(b) | B1(b-1) | A2(b) | B2(b-1)
            ctxs = {}
            ctxs[0] = stageA(0)
            stageA2(0, ctxs[0])
            for b in range(1, BPC):
                ctxs[b] = stageA(b)
                stageB(b - 1, ctxs[b - 1])
                stageA2(b, ctxs[b])
                stageB2(b - 1, ctxs.pop(b - 1))
            stageB(BPC - 1, ctxs[BPC - 1])
            stageB2(BPC - 1, ctxs.pop(BPC - 1))

            # ---------------- head (all 4 batches at once) ----------------
            aggf4 = const.tile([BPC, 2 * D], F32)
            nc.vector.tensor_tensor(aggf4, AGG, fcw2, op=ALU.mult)
            chT = const.tile([BPC, 2], F32)
            nc.vector.tensor_reduce(chT, aggf4[:, :].rearrange("p (k d) -> p k d", k=2),
                                    axis=AX.X, op=ALU.add)
            nc.vector.tensor_scalar(chT, chT, fcb4[:, :], None, op0=ALU.add)
            mask4 = const.tile([BPC, 1], U8)
            nc.vector.tensor_scalar(mask4, totJ, 0.5, None, op0=ALU.is_lt)
            nc.vector.copy_predicated(chT[:, 0:1], mask4, neginf4)
            # softmax over the 2 paths
            mx4 = const.tile([BPC, 1], F32)
            nc.vector.tensor_reduce(mx4, chT, axis=AX.X, op=ALU.max, negate=True)
            ex4 = const.tile([BPC, 2], F32)
            nc.scalar.activation(out=ex4, in_=chT, func=AF.Exp, bias=mx4[:, :], scale=1.0)
            sum4 = const.tile([BPC, 1], F32)
            nc.vector.reduce_sum(sum4, ex4, axis=AX.X)
            rec4 = const.tile([BPC, 1], F32)
            nc.vector.reciprocal(rec4, sum4)
            pp4 = const.tile([BPC, 2], F32)
            nc.vector.tensor_scalar_mul(pp4, ex4, rec4[:, :])
            nc.sync.dma_start(out=pp_d[:, :], in_=pp4)
            # final[b,:] = pp[b,0]*neg_agg + pp[b,1]*pos_agg  (no DMA hop, no matmul)
            AGGw4 = const.tile([BPC, 2 * D], F32)
            ppb = bass.AP(tensor=pp4[...].tensor, offset=pp4[...].offset,
                          ap=[pp4[...].ap[0], [1, 2], [0, D]])
            nc.vector.tensor_tensor(AGGw4[:, :].rearrange("p (k d) -> p k d", k=2),
                                    AGG[:, :].rearrange("p (k d) -> p k d", k=2),
                                    ppb, op=ALU.mult)
            final_s = const.tile([BPC, D], F32)
            nc.vector.tensor_tensor(final_s, AGGw4[:, 0:D], AGGw4[:, D:2 * D], op=ALU.add)
            finT = const.tile([128, NDC, BPC], F32)
            for c in range(NDC):
                pt = ps_tr.tile([128, BPC], F32, tag="tr")
                nc.tensor.transpose(pt, final_s[:, c * 128:(c + 1) * 128], ident[:BPC, :BPC])
                nc.vector.tensor_copy(finT[:, c, :], pt)
            hcl = const.tile([128, NHC, BPC], F32)
            for hc in range(NHC):
                ph = ps_big.tile([128, BPC], F32, tag="big")
                for c in range(NDC):
                    nc.tensor.matmul(ph, cw1_s[:, c, hc * 128:(hc + 1) * 128], finT[:, c, :],
                                     start=(c == 0), stop=(c == NDC - 1))
                nc.scalar.activation(out=hcl[:, hc, :], in_=ph, func=AF.Relu,
                                     bias=cb1c[:, hc:hc + 1], scale=1.0)
            ps_lg = ps_row.tile([BPC, 2], F32, tag="row")
            for hc in range(NHC):
                nc.tensor.matmul(ps_lg, hcl[:, hc, :], cw2_s[:, hc, :],
                                 start=(hc == 0), stop=(hc == NHC - 1))
            lg_s = const.tile([BPC, 2], F32)
            nc.vector.tensor_tensor(lg_s, ps_lg, cb2b, op=ALU.add)
            nc.sync.dma_start(out=logits_d[:, :], in_=lg_s)

    nc.compile()
    return nc


_NC_CACHE = None


def kernel(**inputs):
    global _NC_CACHE
    if _NC_CACHE is None:
        _NC_CACHE = build_nc()
    nc = _NC_CACHE

    word = np.ascontiguousarray(np.asarray(inputs["word_emb"], dtype=np.float32))
    region = np.ascontiguousarray(np.asarray(inputs["region_emb"], dtype=np.float32))
    amask = np.ascontiguousarray(np.asarray(inputs["attention_mask"], dtype=np.int32))
    params = {k: np.ascontiguousarray(np.asarray(inputs[k], dtype=np.float32))
              for k in ["pw1", "pb1", "pw2", "pb2", "nw1", "nb1", "nw2", "nb2",
                        "fcw", "fcb", "cw1", "cb1", "cw2", "cb2"]}

    in_maps = []
    for c in range(NCORES):
        m = {"word_emb": word[c * BPC:(c + 1) * BPC],
             "region_emb": region[c * BPC:(c + 1) * BPC],
             "attention_mask": amask[c * BPC:(c + 1) * BPC]}
        m.update(params)
        in_maps.append(m)

    res = run_bass_kernel_spmd(nc, in_maps, list(range(NCORES)))
    logits = np.concatenate([res.results[c]["logits"] for c in range(NCORES)], axis=0)
    path_prob = np.concatenate([res.results[c]["path_prob"] for c in range(NCORES)], axis=0)
    return (logits.astype(np.float32), path_prob.astype(np.float32))


if __name__ == "__main__":
    rng = np.random.default_rng(0)
    ins = dict(np.load("/root/problem/ref_inputs.npz"))
    out = kernel(**ins)
    print("logits", out[0].shape, "pp", out[1].shape)


# revision 55
# speedup vs baseline: 1.0424x; 1.0424x over previous
"""Trainium2 Bass kernel for nn_DetectionModule (cross-attn + pos/neg expert MLPs).

Self-contained: hardcodes shapes B=32, T=128, R=49, D=512, H=256.
Sharding: pure data-parallel, 4 batches per core across 8 cores.

Key algebraic factorization (avoids materializing [B,T,R,D] = 1.6 GB):
  all_emb @ nw1       = (word@nw1)[t] + (region@nw1)[r]        (broadcast sum)
  einsum(btr,btrd->bd) = sum_t s.sum(r)[t]*word[t] + sum_r s.sum(t)[r]*region[r]

Precision: fp32 everywhere except the neg-expert H pipeline
(wn/rn broadcast-add, tanh, nw2 contraction) which runs in fp16
(HW-measured end-to-end rel err ~1.2e-4 vs jax fp32 reference).

Schedule: engines execute their instruction streams in order, so the
batches are software-pipelined at emission: A1(b)=loads/transposes/attn/
softmax/wn/rn | B1(b-1)=pos path+contraction | A2(b)=H adds+tanh |
B2(b-1)=sigma/mask/reductions/aggs. The nw2 contraction packs 4 psum rows
per bank via tile_position col-groups (zero-padded 32-wide weights) and
ACT evicts them with the sigma-tanh fused; 9 of 49 r-columns' adds run
on GPSIMD in a separate tile to offload the DVE. The head computes choose/softmax/final on a [4, 2*D] packed AGG layout
(no cross-partition DMA hops). Cost-model timeline: ~113us/core (from
184us naive ordering).
"""
import sys

if "/opt/trn_rl_repo" not in sys.path:
    sys.path.insert(0, "/opt/trn_rl_repo")

import numpy as np

import concourse.bass as bass
import concourse.bacc as bacc
import concourse.tile as tile
from concourse import masks, mybir
from concourse.bass_utils import run_bass_kernel_spmd

F32 = mybir.dt.float32
F16 = mybir.dt.float16
U8 = mybir.dt.uint8
I32 = mybir.dt.int32
AF = mybir.ActivationFunctionType
ALU = mybir.AluOpType
AX = mybir.AxisListType

B, T, R, D, H = 32, 128, 49, 512, 256
NCORES = 8
BPC = B // NCORES  # batches per core = 4
NDC = D // 128     # d chunks = 4
NHC = H // 128     # h chunks = 2
RT = R * T         # 6272
# r-blocks for the contraction: 12 blocks of 4 + 1 block of 1 (psum bank = 512 f32)
RBLKS = [(i * 4, 4) for i in range(12)] + [(48, 1)]


def _ap(t, dims, offset=0):
    """AP on tile t keeping its partition dim, custom free dims [[step,count],...]."""
    a = t[...] if not isinstance(t, bass.AP) else t
    return bass.AP(tensor=a.tensor, offset=a.offset + offset, ap=[a.ap[0]] + dims)


def build_nc():
    nc = bacc.Bacc(None, target_bir_lowering=False)

    word_d = nc.dram_tensor("word_emb", [BPC, T, D], F32, kind="ExternalInput")
    region_d = nc.dram_tensor("region_emb", [BPC, R, D], F32, kind="ExternalInput")
    mask_d = nc.dram_tensor("attention_mask", [BPC, T], I32, kind="ExternalInput")
    pw1_d = nc.dram_tensor("pw1", [D, H], F32, kind="ExternalInput")
    pb1_d = nc.dram_tensor("pb1", [H], F32, kind="ExternalInput")
    pw2_d = nc.dram_tensor("pw2", [H, 1], F32, kind="ExternalInput")
    pb2_d = nc.dram_tensor("pb2", [1], F32, kind="ExternalInput")
    nw1_d = nc.dram_tensor("nw1", [D, H], F32, kind="ExternalInput")
    nb1_d = nc.dram_tensor("nb1", [H], F32, kind="ExternalInput")
    nw2_d = nc.dram_tensor("nw2", [H, 1], F32, kind="ExternalInput")
    nb2_d = nc.dram_tensor("nb2", [1], F32, kind="ExternalInput")
    fcw_d = nc.dram_tensor("fcw", [D, 1], F32, kind="ExternalInput")
    fcb_d = nc.dram_tensor("fcb", [1], F32, kind="ExternalInput")
    cw1_d = nc.dram_tensor("cw1", [D, H], F32, kind="ExternalInput")
    cb1_d = nc.dram_tensor("cb1", [H], F32, kind="ExternalInput")
    cw2_d = nc.dram_tensor("cw2", [H, 2], F32, kind="ExternalInput")
    cb2_d = nc.dram_tensor("cb2", [2], F32, kind="ExternalInput")
    logits_d = nc.dram_tensor("logits", [BPC, 2], F32, kind="ExternalOutput")
    pp_d = nc.dram_tensor("path_prob", [BPC, 2], F32, kind="ExternalOutput")

    with tile.TileContext(nc) as tc:
        with (
            tc.tile_pool(name="const", bufs=1) as const,
            tc.tile_pool(name="io", bufs=3) as io,
            tc.tile_pool(name="tr", bufs=3) as tr,
            tc.tile_pool(name="med", bufs=4) as med,
            tc.tile_pool(name="small", bufs=4) as small,
            tc.tile_pool(name="hpool", bufs=6) as hpool,
            tc.tile_pool(name="ps_tr", bufs=2, space="PSUM") as ps_tr,
            tc.tile_pool(name="ps_big", bufs=4, space="PSUM") as ps_big,
            tc.tile_pool(name="ps_row", bufs=2, space="PSUM") as ps_row,
        ):
            # ---------------- early input loads (cut prologue latency) ----------------
            preload = {}
            for b in range(2):
                word_s = io.tile([T, D], F32, tag="word", name=f"word{b}")
                nc.sync.dma_start(out=word_s, in_=word_d[b, :, :])
                region_s = io.tile([R, D], F32, tag="region", name=f"region{b}")
                nc.sync.dma_start(out=region_s, in_=region_d[b, :, :])
                mask_i = small.tile([T, 1], I32, tag="mask_i", name=f"mask_i{b}")
                nc.sync.dma_start(out=mask_i, in_=mask_d[b, :].rearrange("(t o) -> t o", o=1))
                preload[b] = (word_s, region_s, mask_i)

            # ---------------- global constants / params ----------------
            ident = const.tile([128, 128], F32)
            masks.make_identity(nc, ident[:, :])
            ones_col = const.tile([128, 1], F32)
            nc.vector.memset(ones_col, 1.0)
            neginf4 = const.tile([BPC, 1], F32)
            nc.vector.memset(neginf4, -1e9)


            nw1_s = const.tile([128, NDC, H], F32)
            nc.sync.dma_start(out=nw1_s, in_=nw1_d[:, :].rearrange("(c p) h -> p c h", p=128))
            nb1c = const.tile([128, NHC], F32)
            nc.sync.dma_start(out=nb1c, in_=nb1_d[:].rearrange("(c p) -> p c", p=128))
            pw1_s = const.tile([128, NDC, H], F32)
            nc.sync.dma_start(out=pw1_s, in_=pw1_d[:, :].rearrange("(c p) h -> p c h", p=128))
            cw1_s = const.tile([128, NDC, H], F32)
            nc.sync.dma_start(out=cw1_s, in_=cw1_d[:, :].rearrange("(c p) h -> p c h", p=128))

            pw2_s = const.tile([128, NHC], F32)
            nc.sync.dma_start(out=pw2_s, in_=pw2_d[:, :].rearrange("(c p) o -> p (c o)", p=128))
            nw2_s = const.tile([128, NHC], F32)
            nc.sync.dma_start(out=nw2_s, in_=nw2_d[:, :].rearrange("(c p) o -> p (c o)", p=128))
            nw2_16 = const.tile([128, NHC], F16)
            nc.vector.tensor_copy(nw2_16, nw2_s)
            nw2pad = const.tile([128, NHC, 32], F16)
            nc.vector.memset(nw2pad, 0.0)
            for hc in range(NHC):
                nc.vector.tensor_copy(nw2pad[:, hc, 0:1], nw2_s[:, hc:hc + 1])
            cw2_s = const.tile([128, NHC, 2], F32)
            nc.sync.dma_start(out=cw2_s, in_=cw2_d[:, :].rearrange("(c p) o -> p c o", p=128))

            pb1c = const.tile([128, NHC], F32)
            nc.sync.dma_start(out=pb1c, in_=pb1_d[:].rearrange("(c p) -> p c", p=128))
            cb1c = const.tile([128, NHC], F32)
            nc.sync.dma_start(out=cb1c, in_=cb1_d[:].rearrange("(c p) -> p c", p=128))

            # scalar biases broadcast to partitions; pb2h/nb2h = 0.5*bias (sigmoid-via-tanh)
            pb2b = const.tile([128, 1], F32)
            nc.sync.dma_start(out=pb2b, in_=bass.AP(tensor=pb2_d, offset=0, ap=[[0, 128], [1, 1]]))
            pb2h = const.tile([128, 1], F32)
            nc.vector.tensor_scalar_mul(pb2h, pb2b, 0.5)
            nb2b = const.tile([128, 1], F32)
            nc.sync.dma_start(out=nb2b, in_=bass.AP(tensor=nb2_d, offset=0, ap=[[0, 128], [1, 1]]))
            nb2h = const.tile([128, 1], F32)
            nc.vector.tensor_scalar_mul(nb2h, nb2b, 0.5)

            fcw2 = const.tile([BPC, 2 * D], F32)
            nc.sync.dma_start(out=fcw2, in_=bass.AP(tensor=fcw_d, offset=0,
                                                    ap=[[0, BPC], [0, 2], [1, D]]))
            fcb4 = const.tile([BPC, 1], F32)
            nc.sync.dma_start(out=fcb4, in_=bass.AP(tensor=fcb_d, offset=0, ap=[[0, BPC], [1, 1]]))
            cb2b = const.tile([BPC, 2], F32)
            nc.sync.dma_start(out=cb2b, in_=bass.AP(tensor=cb2_d, offset=0, ap=[[0, BPC], [1, 2]]))

            AGG = const.tile([BPC, 2 * D], F32)      # row b = [neg_agg(b) | pos_agg(b)]
            totJ = const.tile([BPC, 1], F32)         # per-batch sum of attn_neg

            # ---------------- per-batch pipeline (2-stage software pipeline) ----------------
            def stageA(b):
                """loads, transposes, attention+softmax, wn/rn, H adds + tanh"""
                ctx = {}
                if b in preload:
                    word_s, region_s, mask_i = preload[b]
                else:
                    word_s = io.tile([T, D], F32, tag="word", name=f"word{b}")
                    nc.sync.dma_start(out=word_s, in_=word_d[b, :, :])
                    region_s = io.tile([R, D], F32, tag="region", name=f"region{b}")
                    nc.sync.dma_start(out=region_s, in_=region_d[b, :, :])
                    mask_i = small.tile([T, 1], I32, tag="mask_i", name=f"mask_i{b}")
                    nc.sync.dma_start(out=mask_i, in_=mask_d[b, :].rearrange("(t o) -> t o", o=1))
                mask_c = small.tile([T, 1], F32, tag="mask_c", name=f"mask_c{b}")
                nc.gpsimd.tensor_copy(mask_c, mask_i)

                wordT32 = tr.tile([128, NDC, T], F32, tag="wordT32", name=f"wordT32_{b}")
                for c in range(NDC):
                    pt = ps_tr.tile([128, T], F32, tag="tr", name=f"ptw{b}_{c}")
                    nc.tensor.transpose(pt, word_s[:, c * 128:(c + 1) * 128], ident[:, :])
                    nc.vector.tensor_copy(wordT32[:, c, :], pt)
                regionT32 = tr.tile([128, NDC, R], F32, tag="regionT32", name=f"regionT32_{b}")
                for c in range(NDC):
                    pt = ps_tr.tile([128, R], F32, tag="tr", name=f"ptr{b}_{c}")
                    nc.tensor.transpose(pt, region_s[:, c * 128:(c + 1) * 128], ident[:R, :R])
                    nc.vector.tensor_copy(regionT32[:, c, :], pt)

                # attention [T, R] fp32
                ps_attn = ps_big.tile([T, R], F32, tag="big", name=f"psattn{b}")
                for c in range(NDC):
                    nc.tensor.matmul(ps_attn, wordT32[:, c, :], regionT32[:, c, :],
                                     start=(c == 0), stop=(c == NDC - 1))
                attn_s = small.tile([T, R], F32, tag="attn", name=f"attn{b}")
                nc.vector.tensor_copy(attn_s, ps_attn)
                negf = small.tile([T, R], F32, tag="negf", name=f"negf{b}")
                nc.gpsimd.tensor_scalar(negf, attn_s, 0.1, None, op0=ALU.is_gt)
                negu = small.tile([T, R], U8, tag="negu", name=f"negu{b}")
                nc.gpsimd.tensor_scalar(negu, attn_s, 0.1, None, op0=ALU.is_gt)
                rowneg = small.tile([T, 1], F32, tag="rowneg", name=f"rowneg{b}")
                nc.vector.reduce_sum(rowneg, negf, axis=AX.X)
                m4 = small.tile([T, R], F32, tag="m4", name=f"m4_{b}")
                nc.gpsimd.tensor_scalar_mul(m4, attn_s, 4.0)
                msk = small.tile([T, R], F32, tag="msk", name=f"msk{b}")
                nc.vector.memset(msk, -4e9)
                nc.vector.copy_predicated(msk, negu, m4)
                mx = small.tile([T, 1], F32, tag="mx", name=f"mx{b}")
                nc.vector.tensor_reduce(mx, msk, axis=AX.X, op=ALU.max, negate=True)
                ex = small.tile([T, R], F32, tag="ex", name=f"ex{b}")
                nc.scalar.activation(out=ex, in_=msk, func=AF.Exp, bias=mx[:, :], scale=1.0)
                sume = small.tile([T, 1], F32, tag="sume", name=f"sume{b}")
                nc.vector.reduce_sum(sume, ex, axis=AX.X)
                rec = small.tile([T, 1], F32, tag="rec", name=f"rec{b}")
                nc.vector.reciprocal(rec, sume)
                attn_pos = small.tile([T, R], F32, tag="attn_pos", name=f"attn_pos{b}")
                nc.gpsimd.tensor_scalar_mul(attn_pos, ex, rec[:, :])

                # wn/rn (fp32 matmuls, fp16/fp32 evictions)
                wnT16 = med.tile([128, NHC, T], F16, tag="wnT16", name=f"wnT16_{b}")
                rnT = med.tile([128, NHC, R], F32, tag="rnT", name=f"rnT{b}")
                for hc in range(NHC):
                    pw = ps_big.tile([128, T], F32, tag="big", name=f"pswn{b}_{hc}")
                    for c in range(NDC):
                        nc.tensor.matmul(pw, nw1_s[:, c, hc * 128:(hc + 1) * 128], wordT32[:, c, :],
                                         start=(c == 0), stop=(c == NDC - 1))
                    nc.vector.tensor_copy(wnT16[:, hc, :], pw)
                    pr = ps_tr.tile([128, R], F32, tag="tr", name=f"psrn{b}_{hc}")
                    for c in range(NDC):
                        nc.tensor.matmul(pr, nw1_s[:, c, hc * 128:(hc + 1) * 128], regionT32[:, c, :],
                                         start=(c == 0), stop=(c == NDC - 1))
                    nc.vector.tensor_scalar(rnT[:, hc, :], pr, nb1c[:, hc:hc + 1], None, op0=ALU.add)

                # H16[h, r*T+t] = tanh(wnT16[h,t] + rnT[h,r])  (emitted in stageA2)
                H16 = []
                H16p = []
                for _hc in range(NHC):
                    Htile = hpool.tile([128, 40 * T], F16, tag="H", name=f"H{b}_{_hc}")
                    H16.append(Htile)
                    Hptile = hpool.tile([128, 9 * T], F16, tag="Hp", name=f"Hp{b}_{_hc}")
                    H16p.append(Hptile)

                ctx.update(word_s=word_s, region_s=region_s, mask_c=mask_c, negf=negf,
                           rowneg=rowneg, attn_pos=attn_pos, H16=H16, H16p=H16p,
                           wnT16=wnT16, rnT=rnT)
                return ctx

            def stageA2(b, ctx, piecewise=False):
                H16, H16p, wnT16, rnT = ctx["H16"], ctx["H16p"], ctx["wnT16"], ctx["rnT"]
                # Pool adds first (big broadcast TT per h-chunk; runs alongside DVE)
                for hc in range(NHC):
                    for r in range(40, 49):
                        nc.gpsimd.tensor_scalar(
                            H16p[hc][:, (r - 40) * T:(r - 39) * T], wnT16[:, hc, :],
                            rnT[:, hc, r:r + 1], None, op0=ALU.add)
                for hc in range(NHC):
                    for r in range(40):
                        nc.vector.tensor_scalar(
                            H16[hc][:, r * T:(r + 1) * T], wnT16[:, hc, :],
                            rnT[:, hc, r:r + 1], None, op0=ALU.add)
                    nc.scalar.activation(out=H16[hc][:, :], in_=H16[hc][:, :], func=AF.Tanh)
                    nc.scalar.activation(out=H16p[hc][:, :], in_=H16p[hc][:, :], func=AF.Tanh)

            def stageB(b, ctx):
                """pos path, contraction, postproc, aggregations"""
                word_s, region_s, mask_c = ctx["word_s"], ctx["region_s"], ctx["mask_c"]
                negf, rowneg, attn_pos = ctx["negf"], ctx["rowneg"], ctx["attn_pos"]
                H16, H16p = ctx["H16"], ctx["H16p"]

                # pos path
                ps_apT = ps_tr.tile([R, T], F32, tag="tr", name=f"psapT{b}")
                nc.tensor.transpose(ps_apT, attn_pos[:, :], ident[:, :])
                apT = small.tile([R, T], F32, tag="apT", name=f"apT{b}")
                nc.vector.tensor_copy(apT, ps_apT)
                ps_pos = ps_big.tile([T, D], F32, tag="big", name=f"pspos{b}")
                nc.tensor.matmul(ps_pos, apT[:, :], region_s[:, :], start=True, stop=True)
                pos_s = med.tile([T, D], F32, tag="pos", name=f"pos{b}")
                nc.vector.tensor_tensor(pos_s, ps_pos, word_s, op=ALU.add)
                posT = tr.tile([128, NDC, T], F32, tag="posT", name=f"posT{b}")
                for c in range(NDC):
                    pt = ps_tr.tile([128, T], F32, tag="tr", name=f"ptp{b}_{c}")
                    nc.tensor.transpose(pt, pos_s[:, c * 128:(c + 1) * 128], ident[:, :])
                    nc.vector.tensor_copy(posT[:, c, :], pt)
                hpT = med.tile([128, NHC, T], F32, tag="hpT", name=f"hpT{b}")
                for hc in range(NHC):
                    ph = ps_big.tile([128, T], F32, tag="big", name=f"pshp{b}_{hc}")
                    for c in range(NDC):
                        nc.tensor.matmul(ph, pw1_s[:, c, hc * 128:(hc + 1) * 128], posT[:, c, :],
                                         start=(c == 0), stop=(c == NDC - 1))
                    nc.scalar.activation(out=hpT[:, hc, :], in_=ph, func=AF.Tanh,
                                         bias=pb1c[:, hc:hc + 1], scale=1.0)
                ps_sp = ps_row.tile([T, 1], F32, tag="row", name=f"pssp{b}")
                for hc in range(NHC):
                    nc.tensor.matmul(ps_sp, hpT[:, hc, :], pw2_s[:, hc:hc + 1],
                                     start=(hc == 0), stop=(hc == NHC - 1))
                sct = small.tile([T, 1], F32, tag="sct", name=f"sct{b}")
                nc.scalar.activation(out=sct, in_=ps_sp, func=AF.Tanh,
                                     bias=pb2h[:, :], scale=0.5)
                score = small.tile([T, 1], F32, tag="score", name=f"score{b}")
                nc.gpsimd.tensor_scalar(score, sct, 0.5, 0.5, op0=ALU.mult, op1=ALU.add)
                ind = small.tile([T, 1], F32, tag="ind", name=f"ind{b}")
                nc.gpsimd.tensor_scalar(ind, rowneg, 0.5, None, op0=ALU.is_gt)
                indm = small.tile([T, 1], F32, tag="indm", name=f"indm{b}")
                nc.gpsimd.tensor_tensor(indm, ind, mask_c, op=ALU.mult)
                score_m = small.tile([T, 1], F32, tag="score_m", name=f"score_m{b}")
                nc.gpsimd.tensor_tensor(score_m, score, indm, op=ALU.mult)
                ps_pagg = ps_row.tile([1, D], F32, tag="row", name=f"pspagg{b}")
                nc.tensor.matmul(ps_pagg, score_m[:, :], pos_s[:, :], start=True, stop=True)
                pagg_row = small.tile([1, D], F32, tag="pagg_row", name=f"pagg{b}")
                nc.vector.tensor_copy(pagg_row, ps_pagg)
                nc.sync.dma_start(out=AGG[b:b + 1, D:2 * D], in_=pagg_row)

                # contraction via col groups; ACT evicts with fused sigma-tanh
                sgT = small.tile([R, T], F32, tag="sgT", name=f"sgT{b}")
                for g in range(3):
                    psg = ps_big.tile([128, 4 * T], F32, tag="big", name=f"psg{b}_{g}")
                    for j in range(4):
                        r0 = 16 * g + 4 * j
                        for hc in range(NHC):
                            if r0 < 40:
                                rhs = H16[hc][:, r0 * T:(r0 + 4) * T]
                            else:
                                rhs = H16p[hc][:, (r0 - 40) * T:(r0 - 36) * T]
                            nc.tensor.matmul(psg[32 * j:32 * j + 32, :], nw2pad[:, hc, :],
                                             rhs,
                                             start=(hc == 0), stop=(hc == NHC - 1),
                                             tile_position=(0, 32 * j))
                    evg = small.tile([128, 4 * T], F32, tag="evg", name=f"evg{b}_{g}")
                    nc.scalar.activation(out=evg, in_=psg, func=AF.Tanh,
                                         bias=nb2h[:, :], scale=0.5)
                    src = bass.AP(tensor=evg[...].tensor, offset=evg[...].offset,
                                  ap=[[4 * T * 32, 4], [1, 4 * T]])
                    nc.sync.dma_start(out=sgT[16 * g:16 * g + 16, :], in_=src)
                prow = ps_row.tile([1, T], F32, tag="row", name=f"prow{b}")
                for hc in range(NHC):
                    nc.tensor.matmul(prow[0:1, :], nw2_16[:, hc:hc + 1],
                                     H16p[hc][:, 8 * T:9 * T],
                                     start=(hc == 0), stop=(hc == NHC - 1))
                tail = small.tile([1, T], F32, tag="tailrow", name=f"tail{b}")
                nc.scalar.activation(out=tail, in_=prow, func=AF.Tanh,
                                     bias=nb2h[0:1, :], scale=0.5)
                nc.sync.dma_start(out=sgT[48:49, :], in_=tail[0:1, :])
                ctx["sgT"] = sgT

            def stageB2(b, ctx):
                word_s, region_s, mask_c = ctx["word_s"], ctx["region_s"], ctx["mask_c"]
                negf, rowneg = ctx["negf"], ctx["rowneg"]
                sgT = ctx["sgT"]
                # sigma finishing + mask + reductions in [R, T]
                sg = small.tile([R, T], F32, tag="sg", name=f"sg{b}")
                nc.gpsimd.tensor_scalar(sg, sgT, 0.5, 0.5, op0=ALU.mult, op1=ALU.add)
                nmf = small.tile([T, R], F32, tag="nmf", name=f"nmf{b}")
                nc.gpsimd.tensor_scalar(nmf, negf, -1.0, 1.0, op0=ALU.mult, op1=ALU.add)
                nmf2 = small.tile([T, R], F32, tag="nmf2", name=f"nmf2_{b}")
                nc.gpsimd.tensor_scalar_mul(nmf2, nmf, mask_c[:, :])
                ps_nm = ps_tr.tile([R, T], F32, tag="tr", name=f"psnm{b}")
                nc.tensor.transpose(ps_nm, nmf2[:, :], ident[:, :])
                nm_T = small.tile([R, T], F32, tag="nm_T", name=f"nm_T{b}")
                nc.vector.tensor_copy(nm_T, ps_nm)
                s_fin = small.tile([R, T], F32, tag="s_fin", name=f"s_fin{b}")
                nc.vector.tensor_tensor(s_fin, sg, nm_T, op=ALU.mult)
                sr_s = small.tile([R, 1], F32, tag="sr_s", name=f"sr{b}")
                nc.vector.reduce_sum(sr_s, s_fin, axis=AX.X)
                ps_sw = ps_row.tile([T, 1], F32, tag="row", name=f"pssw{b}")
                nc.tensor.matmul(ps_sw, s_fin[:, :], ones_col[:R, :], start=True, stop=True)
                sw = small.tile([T, 1], F32, tag="sw", name=f"sw{b}")
                nc.vector.tensor_copy(sw, ps_sw)
                ps_nagg = ps_row.tile([1, D], F32, tag="row", name=f"psnagg{b}")
                nc.tensor.matmul(ps_nagg, sw[:, :], word_s[:, :], start=True, stop=False)
                nc.tensor.matmul(ps_nagg, sr_s[:, :], region_s[:, :], start=False, stop=True)
                nagg_row = small.tile([1, D], F32, tag="nagg_row", name=f"nagg{b}")
                nc.vector.tensor_copy(nagg_row, ps_nagg)
                nc.sync.dma_start(out=AGG[b:b + 1, 0:D], in_=nagg_row)

                ps_tot = ps_row.tile([1, 1], F32, tag="row", name=f"pstot{b}")
                nc.tensor.matmul(ps_tot, rowneg[:, :], ones_col[:, :], start=True, stop=True)
                tot1 = small.tile([1, 1], F32, tag="tot1", name=f"tot1_{b}")
                nc.vector.tensor_copy(tot1, ps_tot)
                nc.sync.dma_start(out=totJ[b:b + 1, :], in_=tot1[0:1, :])

            # pipelined emission: A1(b) | B1(b-1) | A2(b) | B2(b-1)
            ctxs = {}
            ctxs[0] = stageA(0)
            stageA2(0, ctxs[0])
            for b in range(1, BPC):
                ctxs[b] = stageA(b)
                stageB(b - 1, ctxs[b - 1])
                stageA2(b, ctxs[b])
                stageB2(b - 1, ctxs.pop(b - 1))
            stageB(BPC - 1, ctxs[BPC - 1])
            stageB2(BPC - 1, ctxs.pop(BPC - 1))

            # ---------------- head (all 4 batches at once) ----------------
            aggf4 = const.tile([BPC, 2 * D], F32)
            nc.vector.tensor_tensor(aggf4, AGG, fcw2, op=ALU.mult)
            chT = const.tile([BPC, 2], F32)
            nc.vector.tensor_reduce(chT, aggf4[:, :].rearrange("p (k d) -> p k d", k=2),
                                    axis=AX.X, op=ALU.add)
            nc.vector.tensor_scalar(chT, chT, fcb4[:, :], None, op0=ALU.add)
            mask4 = const.tile([BPC, 1], U8)
            nc.vector.tensor_scalar(mask4, totJ, 0.5, None, op0=ALU.is_lt)
            nc.vector.copy_predicated(chT[:, 0:1], mask4, neginf4)
            # softmax over the 2 paths
            mx4 = const.tile([BPC, 1], F32)
            nc.vector.tensor_reduce(mx4, chT, axis=AX.X, op=ALU.max, negate=True)
            ex4 = const.tile([BPC, 2], F32)
            nc.scalar.activation(out=ex4, in_=chT, func=AF.Exp, bias=mx4[:, :], scale=1.0)
            sum4 = const.tile([BPC, 1], F32)
            nc.vector.reduce_sum(sum4, ex4, axis=AX.X)
            rec4 = const.tile([BPC, 1], F32)
            nc.vector.reciprocal(rec4, sum4)
            pp4 = const.tile([BPC, 2], F32)
            nc.vector.tensor_scalar_mul(pp4, ex4, rec4[:, :])
            nc.sync.dma_start(out=pp_d[:, :], in_=pp4)
            # final[b,:] = pp[b,0]*neg_agg + pp[b,1]*pos_agg  (no DMA hop, no matmul)
            AGGw4 = const.tile([BPC, 2 * D], F32)
            ppb = bass.AP(tensor=pp4[...].tensor, offset=pp4[...].offset,
                          ap=[pp4[...].ap[0], [1, 2], [0, D]])
            nc.vector.tensor_tensor(AGGw4[:, :].rearrange("p (k d) -> p k d", k=2),
                                    AGG[:, :].rearrange("p (k d) -> p k d", k=2),
                                    ppb, op=ALU.mult)
            final_s = const.tile([BPC, D], F32)
            nc.vector.tensor_tensor(final_s, AGGw4[:, 0:D], AGGw4[:, D:2 * D], op=ALU.add)
            finT = const.tile([128, NDC, BPC], F32)
            for c in range(NDC):
                pt = ps_tr.tile([128, BPC], F32, tag="tr")
                nc.tensor.transpose(pt, final_s[:, c * 128:(c + 1) * 128], ident[:BPC, :BPC])
                nc.vector.tensor_copy(finT[:, c, :], pt)
            hcl = const.tile([128, NHC, BPC], F32)
            for hc in range(NHC):
                ph = ps_big.tile([128, BPC], F32, tag="big")
                for c in range(NDC):
                    nc.tensor.matmul(ph, cw1_s[:, c, hc * 128:(hc + 1) * 128], finT[:, c, :],
                                     start=(c == 0), stop=(c == NDC - 1))
                nc.scalar.activation(out=hcl[:, hc, :], in_=ph, func=AF.Relu,
                                     bias=cb1c[:, hc:hc + 1], scale=1.0)
            ps_lg = ps_row.tile([BPC, 2], F32, tag="row")
            for hc in range(NHC):
                nc.tensor.matmul(ps_lg, hcl[:, hc, :], cw2_s[:, hc, :],
                                 start=(hc == 0), stop=(hc == NHC - 1))
            lg_s = const.tile([BPC, 2], F32)
            nc.vector.tensor_tensor(lg_s, ps_lg, cb2b, op=ALU.add)
            nc.sync.dma_start(out=logits_d[:, :], in_=lg_s)

    nc.compile()
    return nc


_NC_CACHE = None


def kernel(**inputs):
    global _NC_CACHE
    if _NC_CACHE is None:
        _NC_CACHE = build_nc()
    nc = _NC_CACHE

    word = np.ascontiguousarray(np.asarray(inputs["word_emb"], dtype=np.float32))
    region = np.ascontiguousarray(np.asarray(inputs["region_emb"], dtype=np.float32))
    amask = np.ascontiguousarray(np.asarray(inputs["attention_mask"], dtype=np.int32))
    params = {k: np.ascontiguousarray(np.asarray(inputs[k], dtype=np.float32))
              for k in ["pw1", "pb1", "pw2", "pb2", "nw1", "nb1", "nw2", "nb2",
                        "fcw", "fcb", "cw1", "cb1", "cw2", "cb2"]}

    in_maps = []
    for c in range(NCORES):
        m = {"word_emb": word[c * BPC:(c + 1) * BPC],
             "region_emb": region[c * BPC:(c + 1) * BPC],
             "attention_mask": amask[c * BPC:(c + 1) * BPC]}
        m.update(params)
        in_maps.append(m)

    res = run_bass_kernel_spmd(nc, in_maps, list(range(NCORES)))
    logits = np.concatenate([res.results[c]["logits"] for c in range(NCORES)], axis=0)
    path_prob = np.concatenate([res.results[c]["path_prob"] for c in range(NCORES)], axis=0)
    return (logits.astype(np.float32), path_prob.astype(np.float32))


if __name__ == "__main__":
    rng = np.random.default_rng(0)
    ins = dict(np.load("/root/problem/ref_inputs.npz"))
    out = kernel(**ins)
    print("logits", out[0].shape, "pp", out[1].shape)
